# revision 1
# baseline (speedup 1.0000x reference)
"""MoE layer (top-2 of 8 experts) on 8 NeuronCores via expert parallelism.

Contract: kernel(**inputs) takes the FULL unsharded inputs
  x [2, 2048, 1024] f32, Wr [1024, 8] f32, W1 [8, 1024, 4096] f32,
  W2 [8, 4096, 1024] f32
and returns the FULL output (out [2, 2048, 1024] f32, aux_loss f32).

Strategy (expert parallelism, per the sharding hint):
- Host computes the (tiny) router: logits = x@Wr, softmax, top-2,
  normalized combine weights.  This decides the sharding: tokens are
  all-to-all'd (gathered) by routed expert on the host.
- Core e receives: its expert's W1[e]/W2[e] (cast bf16), the tokens
  routed to expert e (x^T layout, cast bf16, padded to capacity C),
  and the per-token combine weight.
- Core e computes y = gelu(x @ W1) @ W2 * w entirely on device:
    phase A: h^T[f, m] = gelu(sum_d W1[d, f] * x^T[d, m])  (PE + ACT)
    phase B: y[m, d]   = sum_f h^T[f, m] * W2[f, d]        (PE)
  bf16 operands, fp32 PSUM accumulation, exact (erf) Gelu on fp32.
- Host scatter-adds the weighted per-expert outputs back into the
  full [4096, 1024] output (the "un-all-to-all").
"""

import os
import sys
import types

import numpy as np
import ml_dtypes

# ---------------------------------------------------------------------------
# Optional NTFF-profile hook for trace=True under axon.  concourse's
# run_bass_kernel_spmd imports antenv.axon_hooks unconditionally when tracing
# is requested; agent images lack that module, so synthesize it.
# ---------------------------------------------------------------------------


def _install_axon_hooks():
    if "antenv.axon_hooks" in sys.modules:
        return
    hook = None
    try:
        import trn_agent_boot.trn_boot as _tb

        so = "/opt/axon/libaxon_pjrt.so"
        if os.path.exists(so):
            hook = _tb._ntff_profile_via_ctypes(so)
    except Exception:
        hook = None
    m = types.ModuleType("antenv.axon_hooks")
    m.get_axon_ntff_profile_hook = lambda: hook
    m.set_axon_ntff_profile_hook = lambda h: None
    sys.modules["antenv.axon_hooks"] = m
    try:
        import antenv

        antenv.axon_hooks = m
    except Exception:
        pass


_install_axon_hooks()

import concourse.bass as bass  # noqa: E402
import concourse.tile as tile  # noqa: E402
from concourse import bacc, mybir  # noqa: E402
from concourse.bass_utils import run_bass_kernel_spmd  # noqa: E402

BF16 = mybir.dt.bfloat16
F32 = mybir.dt.float32
NP_BF16 = ml_dtypes.bfloat16

N_CORES = 8
TOP_K = 2
P = 128

# Results of the most recent device run (for the test harness).
LAST_RESULTS = None

_KERNEL_CACHE: dict = {}


# ---------------------------------------------------------------------------
# Device kernel: y[m, d] = gelu(x[m, :] @ W1) @ W2 * w[m]   for one expert
# ---------------------------------------------------------------------------


def build_expert_ffn(C: int, D: int, F: int):
    """Build the per-core expert FFN Bass kernel for capacity C tokens."""
    assert C % P == 0 and D % P == 0 and F % P == 0
    KD = D // P          # k-tiles of matmul 1 (contraction over D)
    FT = F // P          # f-tiles = k-tiles of matmul 2 (contraction over F)
    DN = max(1, D // 512)  # 512-wide output column tiles
    NB = C // P          # 128-row output blocks

    nc = bacc.Bacc(
        "TRN2",
        target_bir_lowering=False,
        debug=False,
        enable_asserts=False,
        num_devices=N_CORES,
    )
    xt = nc.dram_tensor("xt", [P, KD, C], BF16, kind="ExternalInput")
    w1 = nc.dram_tensor("w1", [P, KD, F], BF16, kind="ExternalInput")
    w2 = nc.dram_tensor("w2", [P, FT, D], BF16, kind="ExternalInput")
    wt = nc.dram_tensor("wt", [P, NB], F32, kind="ExternalInput")
    y = nc.dram_tensor("y", [C, D], F32, kind="ExternalOutput")

    # token chunking along C for phase A (moving-dim of matmul 1)
    M_CHUNK = 512
    chunks = []
    m0 = 0
    while m0 < C:
        msz = min(M_CHUNK, C - m0)
        chunks.append((m0, msz))
        m0 += msz

    with tile.TileContext(nc) as tc:
        with (
            tc.tile_pool(name="weights", bufs=1) as wpool,
            tc.tile_pool(name="acts", bufs=1) as apool,
            tc.tile_pool(name="h", bufs=1) as hpool,
            tc.tile_pool(name="psum", bufs=4, space="PSUM") as pspool,
            tc.tile_pool(name="out", bufs=4) as opool,
        ):
            # resident weights / activations (DMA'd in slices so compute can
            # start early and multiple DMA queues engage)
            w1sb = wpool.tile([P, KD, F], BF16)
            for kd in range(KD):
                nc.sync.dma_start(w1sb[:, kd, :], w1.ap()[:, kd, :])
            xtsb = apool.tile([P, KD, C], BF16)
            for kd in range(KD):
                nc.sync.dma_start(xtsb[:, kd, :], xt.ap()[:, kd, :])
            wtsb = apool.tile([P, NB], F32)
            nc.sync.dma_start(wtsb[:], wt.ap())
            w2sb = wpool.tile([P, FT, D], BF16)
            for ft in range(0, FT, 4):
                nc.sync.dma_start(w2sb[:, ft : ft + 4, :], w2.ap()[:, ft : ft + 4, :])

            gelu = mybir.ActivationFunctionType.Gelu
            for m0, msz in chunks:
                # phase A: h^T[f-tile, m] = gelu(W1^T @ x^T) for this chunk
                hT = hpool.tile([P, FT, M_CHUNK], BF16)
                for ft in range(FT):
                    ps = pspool.tile([P, M_CHUNK], F32, tag="h")
                    for kd in range(KD):
                        nc.tensor.matmul(
                            ps[:, :msz],
                            lhsT=w1sb[:, kd, ft * P : (ft + 1) * P],
                            rhs=xtsb[:, kd, m0 : m0 + msz],
                            start=(kd == 0),
                            stop=(kd == KD - 1),
                        )
                    nc.scalar.activation(hT[:, ft, :msz], ps[:, :msz], gelu)

                # phase B: y[m, d] = h @ W2, scaled by per-token weight
                for mc in range(msz // P):
                    blk = m0 // P + mc
                    for dn in range(DN):
                        ps2 = pspool.tile([P, 512], F32, tag="y")
                        for kf in range(FT):
                            nc.tensor.matmul(
                                ps2,
                                lhsT=hT[:, kf, mc * P : (mc + 1) * P],
                                rhs=w2sb[:, kf, dn * 512 : (dn + 1) * 512],
                                start=(kf == 0),
                                stop=(kf == FT - 1),
                            )
                        ysb = opool.tile([P, 512], F32)
                        nc.vector.tensor_scalar_mul(
                            ysb[:], ps2[:], wtsb[:, blk : blk + 1]
                        )
                        nc.sync.dma_start(
                            y.ap()[blk * P : (blk + 1) * P, dn * 512 : (dn + 1) * 512],
                            ysb[:],
                        )
    nc.compile()
    return nc


def get_kernel(C: int, D: int, F: int):
    key = (C, D, F)
    if key not in _KERNEL_CACHE:
        _KERNEL_CACHE[key] = build_expert_ffn(C, D, F)
    return _KERNEL_CACHE[key]


# ---------------------------------------------------------------------------
# Host side: routing, sharding, gather/scatter
# ---------------------------------------------------------------------------


def _route(xf: np.ndarray, Wr: np.ndarray):
    """fp32 router identical to the reference: softmax + top-2 + renorm."""
    logits = xf @ Wr  # [N, E] fp32
    m = logits.max(axis=-1, keepdims=True)
    e = np.exp(logits - m)
    probs = (e / e.sum(axis=-1, keepdims=True)).astype(np.float32)
    # top-2, descending, ties -> lower index (jax.lax.top_k semantics)
    top_i = np.argsort(-probs, axis=-1, kind="stable")[:, :TOP_K]
    tw = np.take_along_axis(probs, top_i, axis=-1)
    top_w = (tw / tw.sum(axis=-1, keepdims=True)).astype(np.float32)
    return top_i, top_w


def kernel(x: np.ndarray, Wr: np.ndarray, W1: np.ndarray, W2: np.ndarray):
    global LAST_RESULTS
    x = np.ascontiguousarray(np.asarray(x, dtype=np.float32))
    Wr = np.ascontiguousarray(np.asarray(Wr, dtype=np.float32))
    W1 = np.ascontiguousarray(np.asarray(W1, dtype=np.float32))
    W2 = np.ascontiguousarray(np.asarray(W2, dtype=np.float32))

    B, T, D = x.shape
    E = Wr.shape[1]
    F = W1.shape[2]
    N = B * T
    xf = x.reshape(N, D)

    # --- router (host, fp32, replicated logic) ---
    top_i, top_w = _route(xf, Wr)

    # --- all-to-all by routed expert: build per-expert token lists ---
    idx_e = [np.where((top_i == e).any(axis=1))[0] for e in range(E)]
    w_e = []
    for e in range(E):
        sel = top_i[idx_e[e]] == e  # [n_e, 2] exactly one True per row
        w_e.append(
            np.where(sel[:, 0], top_w[idx_e[e], 0], top_w[idx_e[e], 1]).astype(
                np.float32
            )
        )
    counts = np.array([len(i) for i in idx_e])
    C = max(P, int(np.ceil(counts.max() / P) * P))  # capacity (static shape)

    KD = D // P
    FT = F // P
    NB = C // P

    # --- per-core input prep (gather + cast + device layout) ---
    in_maps = []
    for e in range(E):
        n_e = counts[e]
        xe = np.zeros((C, D), dtype=np.float32)
        xe[:n_e] = xf[idx_e[e]]
        # x^T arranged [128, KD, C]
        xt_host = np.ascontiguousarray(
            xe.T.reshape(KD, P, C).transpose(1, 0, 2)
        ).astype(NP_BF16)
        w1_host = np.ascontiguousarray(
            W1[e].reshape(KD, P, F).transpose(1, 0, 2)
        ).astype(NP_BF16)
        w2_host = np.ascontiguousarray(
            W2[e].reshape(FT, P, D).transpose(1, 0, 2)
        ).astype(NP_BF16)
        wfull = np.zeros(C, dtype=np.float32)
        wfull[:n_e] = w_e[e]
        wt_host = np.ascontiguousarray(wfull.reshape(NB, P).T)
        in_maps.append({"xt": xt_host, "w1": w1_host, "w2": w2_host, "wt": wt_host})

    # --- run on the 8 cores ---
    nc = get_kernel(C, D, F)
    res = run_bass_kernel_spmd(nc, in_maps, core_ids=list(range(N_CORES)))
    LAST_RESULTS = res

    # --- un-all-to-all: weighted scatter-add (weights already applied) ---
    out = np.zeros((N, D), dtype=np.float32)
    for e in range(E):
        n_e = counts[e]
        if n_e:
            out[idx_e[e]] += res.results[e]["y"][:n_e]

    return out.reshape(B, T, D), np.float32(0.0)


# revision 4
# speedup vs baseline: 1.0916x; 1.0916x over previous
"""MoE layer (top-2 of 8 experts) on 8 NeuronCores via expert parallelism.

Contract: kernel(**inputs) takes the FULL unsharded inputs
  x [2, 2048, 1024] f32, Wr [1024, 8] f32, W1 [8, 1024, 4096] f32,
  W2 [8, 4096, 1024] f32
and returns the FULL output (out [2, 2048, 1024] f32, aux_loss f32).

Strategy (expert parallelism, per the sharding hint):
- Host computes the (tiny) router: logits = x@Wr, softmax, top-2,
  normalized combine weights.  This decides the sharding: tokens are
  all-to-all'd (gathered) by routed expert on the host.
- Core e receives: its expert's W1[e]/W2[e] (cast bf16) and the tokens
  routed to expert e (x^T layout, cast bf16, padded to capacity C).
- Core e computes y^T = (gelu(x @ W1) @ W2)^T entirely on device:
    phase A: h^T[f, m] = gelu(sum_d W1[d, f] * x^T[d, m])  (PE + ACT)
    phase B: y^T[d, m] = sum_f W2[f, d] * h^T[f, m]        (PE)
  bf16 operands, fp32 PSUM accumulation, exact (erf) Gelu on fp32.
  Phase A runs contraction(kd)-outermost over groups of 4 PSUM banks so
  compute starts as soon as the first weight slices land (instead of
  waiting for all of W1).
- Host applies the combine weights and scatter-adds the per-expert
  outputs back into the full [4096, 1024] output (the "un-all-to-all").
"""

import os
import sys
import types

import numpy as np
import ml_dtypes

# ---------------------------------------------------------------------------
# Optional NTFF-profile hook for trace=True under axon.  concourse's
# run_bass_kernel_spmd imports antenv.axon_hooks unconditionally when tracing
# is requested; agent images lack that module, so synthesize it.
# ---------------------------------------------------------------------------


def _install_axon_hooks():
    if "antenv.axon_hooks" in sys.modules:
        return
    hook = None
    try:
        import trn_agent_boot.trn_boot as _tb

        so = "/opt/axon/libaxon_pjrt.so"
        if os.path.exists(so):
            hook = _tb._ntff_profile_via_ctypes(so)
    except Exception:
        hook = None
    m = types.ModuleType("antenv.axon_hooks")
    m.get_axon_ntff_profile_hook = lambda: hook
    m.set_axon_ntff_profile_hook = lambda h: None
    sys.modules["antenv.axon_hooks"] = m
    try:
        import antenv

        antenv.axon_hooks = m
    except Exception:
        pass


_install_axon_hooks()

import concourse.bass as bass  # noqa: E402
import concourse.tile as tile  # noqa: E402
from concourse import bacc, mybir  # noqa: E402
from concourse.bass_utils import run_bass_kernel_spmd  # noqa: E402

BF16 = mybir.dt.bfloat16
F32 = mybir.dt.float32
NP_BF16 = ml_dtypes.bfloat16

N_CORES = 8
TOP_K = 2
P = 128

# Results of the most recent device run (for the test harness).
LAST_RESULTS = None

_KERNEL_CACHE: dict = {}


def _chunks_of(C: int, maxc: int = 512, align: int = 64):
    """Split C into near-even chunks, each <= maxc and a multiple of align."""
    n = -(-C // maxc)
    base = C // n
    base -= base % align
    sizes = [base] * n
    rem = C - base * n
    i = 0
    while rem > 0:
        add = min(align, rem, maxc - sizes[i])
        sizes[i] += add
        rem -= add
        i = (i + 1) % n
    out, m0 = [], 0
    for s in sizes:
        out.append((m0, s))
        m0 += s
    return out


# ---------------------------------------------------------------------------
# Device kernel: y^T[d, m] = (gelu(x[m, :] @ W1) @ W2)^T   for one expert
# ---------------------------------------------------------------------------


def build_expert_ffn(C: int, D: int, F: int):
    """Build the per-core expert FFN Bass kernel for capacity C tokens."""
    assert C % 64 == 0 and D % P == 0 and F % P == 0
    KD = D // P          # k-tiles of matmul 1 (contraction over D)
    FT = F // P          # f-tiles = k-tiles of matmul 2 (contraction over F)
    DP = D // P          # 128-row output blocks of y^T
    FG = 4               # ft-group size (PSUM banks used by phase A)

    nc = bacc.Bacc(
        "TRN2",
        target_bir_lowering=False,
        debug=False,
        enable_asserts=False,
        num_devices=N_CORES,
    )
    xt = nc.dram_tensor("xt", [P, KD, C], BF16, kind="ExternalInput")
    w1 = nc.dram_tensor("w1", [P, KD, F], BF16, kind="ExternalInput")
    w2 = nc.dram_tensor("w2", [P, FT, D], BF16, kind="ExternalInput")
    yt = nc.dram_tensor("yt", [D, C], F32, kind="ExternalOutput")

    chunks = _chunks_of(C)
    CHUNK_MAX = max(s for _, s in chunks)
    W1_COLS = FG * P  # W1 DMA piece width in f-columns

    with tile.TileContext(nc) as tc:
        with (
            tc.tile_pool(name="weights", bufs=1) as wpool,
            tc.tile_pool(name="acts", bufs=1) as apool,
            tc.tile_pool(name="h", bufs=1) as hpool,
            tc.tile_pool(name="psum", bufs=1, space="PSUM") as pspool,
            tc.tile_pool(name="out", bufs=4) as opool,
        ):
            # activations first (small; phase A needs xt[kd=0] immediately)
            xtsb = apool.tile([P, KD, C], BF16)
            for kd in range(KD):
                nc.sync.dma_start(xtsb[:, kd, :], xt.ap()[:, kd, :])
            # W1 in (kd x W1_COLS) pieces, f-group-major so the first
            # ft-groups' weights arrive first
            w1sb = wpool.tile([P, KD, F], BF16)
            for f0 in range(0, F, W1_COLS):
                for kd in range(KD):
                    nc.sync.dma_start(
                        w1sb[:, kd, f0 : f0 + W1_COLS],
                        w1.ap()[:, kd, f0 : f0 + W1_COLS],
                    )
            w2sb = wpool.tile([P, FT, D], BF16)
            for ft in range(0, FT, 4):
                nc.sync.dma_start(w2sb[:, ft : ft + 4, :], w2.ap()[:, ft : ft + 4, :])

            gelu = mybir.ActivationFunctionType.Gelu
            for m0, msz in chunks:
                # phase A: h^T[f, m] = gelu(W1^T @ x^T), kd-outer in groups
                # of FG PSUM banks
                hT = hpool.tile([P, FT, CHUNK_MAX], BF16)
                for g in range(FT // FG):
                    pss = [
                        pspool.tile(
                            [P, 512], F32, tag="a", bufs=FG + 2, name=f"ps_a_{j}"
                        )
                        for j in range(FG)
                    ]
                    for kd in range(KD):
                        for j in range(FG):
                            ft = g * FG + j
                            nc.tensor.matmul(
                                pss[j][:, :msz],
                                lhsT=w1sb[:, kd, ft * P : (ft + 1) * P],
                                rhs=xtsb[:, kd, m0 : m0 + msz],
                                start=(kd == 0),
                                stop=(kd == KD - 1),
                            )
                    for j in range(FG):
                        nc.scalar.activation(
                            hT[:, g * FG + j, :msz], pss[j][:, :msz], gelu
                        )

                # phase B: y^T[d, m] = W2^T-contraction over f (kf-inner)
                for dp in range(DP):
                    ps2 = pspool.tile([P, 512], F32, tag="b", bufs=2, name="ps_b")
                    for kf in range(FT):
                        nc.tensor.matmul(
                            ps2[:, :msz],
                            lhsT=w2sb[:, kf, dp * P : (dp + 1) * P],
                            rhs=hT[:, kf, :msz],
                            start=(kf == 0),
                            stop=(kf == FT - 1),
                        )
                    ysb = opool.tile([P, 512], F32)
                    nc.vector.tensor_copy(ysb[:, :msz], ps2[:, :msz])
                    nc.sync.dma_start(
                        yt.ap()[dp * P : (dp + 1) * P, m0 : m0 + msz], ysb[:, :msz]
                    )
    nc.compile()
    return nc


def get_kernel(C: int, D: int, F: int):
    key = (C, D, F)
    if key not in _KERNEL_CACHE:
        _KERNEL_CACHE[key] = build_expert_ffn(C, D, F)
    return _KERNEL_CACHE[key]


# ---------------------------------------------------------------------------
# Host side: routing, sharding, gather/scatter
# ---------------------------------------------------------------------------


def _route(xf: np.ndarray, Wr: np.ndarray):
    """fp32 router identical to the reference: softmax + top-2 + renorm."""
    logits = xf @ Wr  # [N, E] fp32
    m = logits.max(axis=-1, keepdims=True)
    e = np.exp(logits - m)
    probs = (e / e.sum(axis=-1, keepdims=True)).astype(np.float32)
    # top-2, descending, ties -> lower index (jax.lax.top_k semantics)
    top_i = np.argsort(-probs, axis=-1, kind="stable")[:, :TOP_K]
    tw = np.take_along_axis(probs, top_i, axis=-1)
    top_w = (tw / tw.sum(axis=-1, keepdims=True)).astype(np.float32)
    return top_i, top_w


def kernel(x: np.ndarray, Wr: np.ndarray, W1: np.ndarray, W2: np.ndarray):
    global LAST_RESULTS
    x = np.ascontiguousarray(np.asarray(x, dtype=np.float32))
    Wr = np.ascontiguousarray(np.asarray(Wr, dtype=np.float32))
    W1 = np.ascontiguousarray(np.asarray(W1, dtype=np.float32))
    W2 = np.ascontiguousarray(np.asarray(W2, dtype=np.float32))

    B, T, D = x.shape
    E = Wr.shape[1]
    F = W1.shape[2]
    N = B * T
    xf = x.reshape(N, D)

    # --- router (host, fp32, replicated logic) ---
    top_i, top_w = _route(xf, Wr)

    # --- all-to-all by routed expert: build per-expert token lists ---
    idx_e = [np.where((top_i == e).any(axis=1))[0] for e in range(E)]
    w_e = []
    for e in range(E):
        sel = top_i[idx_e[e]] == e  # [n_e, 2] exactly one True per row
        w_e.append(
            np.where(sel[:, 0], top_w[idx_e[e], 0], top_w[idx_e[e], 1]).astype(
                np.float32
            )
        )
    counts = np.array([len(i) for i in idx_e])
    C = max(128, int(np.ceil(counts.max() / 64) * 64))  # capacity (static shape)

    KD = D // P
    FT = F // P

    # --- per-core input prep (gather + cast + device layout) ---
    in_maps = []
    for e in range(E):
        n_e = counts[e]
        xe = np.zeros((C, D), dtype=np.float32)
        xe[:n_e] = xf[idx_e[e]]
        # x^T arranged [128, KD, C]
        xt_host = np.ascontiguousarray(
            xe.T.reshape(KD, P, C).transpose(1, 0, 2)
        ).astype(NP_BF16)
        w1_host = np.ascontiguousarray(
            W1[e].reshape(KD, P, F).transpose(1, 0, 2)
        ).astype(NP_BF16)
        w2_host = np.ascontiguousarray(
            W2[e].reshape(FT, P, D).transpose(1, 0, 2)
        ).astype(NP_BF16)
        in_maps.append({"xt": xt_host, "w1": w1_host, "w2": w2_host})

    # --- run on the 8 cores ---
    nc = get_kernel(C, D, F)
    res = run_bass_kernel_spmd(nc, in_maps, core_ids=list(range(N_CORES)))
    LAST_RESULTS = res

    # --- un-all-to-all: weighted scatter-add ---
    out = np.zeros((N, D), dtype=np.float32)
    for e in range(E):
        n_e = counts[e]
        if n_e:
            y = res.results[e]["yt"][:, :n_e].T  # [n_e, D]
            out[idx_e[e]] += w_e[e][:, None] * y

    return out.reshape(B, T, D), np.float32(0.0)


# revision 9
# speedup vs baseline: 1.1270x; 1.0324x over previous
"""MoE layer (top-2 of 8 experts) on 8 NeuronCores via expert parallelism.

Contract: kernel(**inputs) takes the FULL unsharded inputs
  x [2, 2048, 1024] f32, Wr [1024, 8] f32, W1 [8, 1024, 4096] f32,
  W2 [8, 4096, 1024] f32
and returns the FULL output (out [2, 2048, 1024] f32, aux_loss f32).

Strategy (expert parallelism, per the sharding hint):
- Host computes the (tiny) router: logits = x@Wr, softmax, top-2,
  normalized combine weights.  This decides the sharding: tokens are
  all-to-all'd (gathered) by routed expert on the host.
- Core e receives: its expert's W1[e]/W2[e] (cast bf16) and the tokens
  routed to expert e (x^T layout, cast bf16, padded to capacity C).
- Core e computes y^T = (gelu(x @ W1) @ W2)^T entirely on device:
    phase A: h^T[f, m] = gelu(sum_d W1[d, f] * x^T[d, m])  (PE + ACT)
    phase B: y^T[d, m] = sum_f W2[f, d] * h^T[f, m]        (PE)
  bf16 operands, fp32 PSUM accumulation, exact (erf) Gelu on fp32.
  Phase A runs contraction(kd)-outermost over groups of 4 PSUM banks so
  compute starts as soon as the first weight slices land (instead of
  waiting for all of W1).
- Host applies the combine weights and scatter-adds the per-expert
  outputs back into the full [4096, 1024] output (the "un-all-to-all").
"""

import os
import sys
import types

import numpy as np
import ml_dtypes

# ---------------------------------------------------------------------------
# Optional NTFF-profile hook for trace=True under axon.  concourse's
# run_bass_kernel_spmd imports antenv.axon_hooks unconditionally when tracing
# is requested; agent images lack that module, so synthesize it.
# ---------------------------------------------------------------------------


def _install_axon_hooks():
    if "antenv.axon_hooks" in sys.modules:
        return
    hook = None
    try:
        import trn_agent_boot.trn_boot as _tb

        so = "/opt/axon/libaxon_pjrt.so"
        if os.path.exists(so):
            hook = _tb._ntff_profile_via_ctypes(so)
    except Exception:
        hook = None
    m = types.ModuleType("antenv.axon_hooks")
    m.get_axon_ntff_profile_hook = lambda: hook
    m.set_axon_ntff_profile_hook = lambda h: None
    sys.modules["antenv.axon_hooks"] = m
    try:
        import antenv

        antenv.axon_hooks = m
    except Exception:
        pass


_install_axon_hooks()

import concourse.bass as bass  # noqa: E402
import concourse.tile as tile  # noqa: E402
from concourse import bacc, mybir  # noqa: E402
from concourse.bass_utils import run_bass_kernel_spmd  # noqa: E402

BF16 = mybir.dt.bfloat16
F32 = mybir.dt.float32
NP_BF16 = ml_dtypes.bfloat16

N_CORES = 8
TOP_K = 2
P = 128

# Results of the most recent device run (for the test harness).
LAST_RESULTS = None

_KERNEL_CACHE: dict = {}


def _chunks_of(C: int, maxc: int = 512, align: int = 64):
    """Split C into chunks <= maxc, multiples of align, first chunk largest
    (the first chunk races the weight DMA, so bigger = more slack)."""
    n = -(-C // maxc)
    sizes = [min(maxc, C)]
    rem = C - sizes[0]
    if rem:
        base = (rem // (n - 1)) // align * align
        tail = [base] * (n - 1)
        left = rem - base * (n - 1)
        i = 0
        while left:
            tail[i] += align
            left -= align
            i = (i + 1) % (n - 1)
        sizes += tail
    out, m0 = [], 0
    for s in sizes:
        out.append((m0, s))
        m0 += s
    return out


# ---------------------------------------------------------------------------
# Device kernel: y^T[d, m] = (gelu(x[m, :] @ W1) @ W2)^T   for one expert
# ---------------------------------------------------------------------------


def build_expert_ffn(C: int, D: int, F: int):
    """Build the per-core expert FFN Bass kernel for capacity C tokens."""
    assert C % 64 == 0 and D % P == 0 and F % P == 0
    KD = D // P          # k-tiles of matmul 1 (contraction over D)
    FT = F // P          # f-tiles = k-tiles of matmul 2 (contraction over F)
    DP = D // P          # 128-row output blocks of y^T
    FG = 4               # ft-group size (PSUM banks used by phase A)

    nc = bacc.Bacc(
        "TRN2",
        target_bir_lowering=False,
        debug=False,
        enable_asserts=False,
        num_devices=N_CORES,
    )
    xt = nc.dram_tensor("xt", [P, KD, C], BF16, kind="ExternalInput")
    w1 = nc.dram_tensor("w1", [P, KD, F], BF16, kind="ExternalInput")
    # W2 repacked per output-row-block: [p, dp, kf, q] = W2[kf*128+p, dp*128+q]
    w2 = nc.dram_tensor("w2", [P, DP, FT, P], BF16, kind="ExternalInput")
    yt = nc.dram_tensor("yt", [D, C], F32, kind="ExternalOutput")

    chunks = _chunks_of(C)
    CHUNK_MAX = max(s for _, s in chunks)
    W1_COLS = FG * P  # fine W1 DMA piece width (f-columns) for the early race

    with tile.TileContext(nc) as tc:
        with (
            tc.tile_pool(name="weights", bufs=1) as wpool,
            tc.tile_pool(name="acts", bufs=1) as apool,
            tc.tile_pool(name="h", bufs=1) as hpool,
            tc.tile_pool(name="psum", bufs=1, space="PSUM") as pspool,
            tc.tile_pool(name="out", bufs=4) as opool,
        ):
            xtsb = apool.tile([P, KD, C], BF16)
            w1sb = wpool.tile([P, KD, F], BF16)
            w2sb = wpool.tile([P, DP, FT, P], BF16)
            m0_0, msz_0 = chunks[0]

            # Early supply race: the first ft-group needs xt[:, kd, chunk0]
            # and w1[:, kd, 0:FG*P] for every kd.  Interleave them on two
            # issue engines (gpsimd=SWDGE, sync=HWDGE) so issue rate doesn't
            # serialize the supply.
            for kd in range(KD):
                nc.gpsimd.dma_start(
                    xtsb[:, kd, m0_0 : m0_0 + msz_0],
                    xt.ap()[:, kd, m0_0 : m0_0 + msz_0],
                )
                nc.sync.dma_start(
                    w1sb[:, kd, 0:W1_COLS], w1.ap()[:, kd, 0:W1_COLS]
                )
            for kd in range(KD):
                nc.sync.dma_start(
                    w1sb[:, kd, W1_COLS : 2 * W1_COLS],
                    w1.ap()[:, kd, W1_COLS : 2 * W1_COLS],
                )
            # first W2 row-block early so phase B chunk0 can start on time
            nc.sync.dma_start(w2sb[:, 0], w2.ap()[:, 0])
            # coarse remainder of W1, W2 row-blocks interleaved
            w2_next = 1
            for f0 in range(2 * W1_COLS, F, 2 * W1_COLS):
                for kd in range(KD):
                    nc.sync.dma_start(
                        w1sb[:, kd, f0 : f0 + 2 * W1_COLS],
                        w1.ap()[:, kd, f0 : f0 + 2 * W1_COLS],
                    )
                for _ in range(2):
                    if w2_next < DP:
                        nc.sync.dma_start(w2sb[:, w2_next], w2.ap()[:, w2_next])
                        w2_next += 1
            while w2_next < DP:
                nc.sync.dma_start(w2sb[:, w2_next], w2.ap()[:, w2_next])
                w2_next += 1
            # later token chunks (needed much later)
            for m0, msz in chunks[1:]:
                nc.gpsimd.dma_start(
                    xtsb[:, :, m0 : m0 + msz], xt.ap()[:, :, m0 : m0 + msz]
                )

            gelu = mybir.ActivationFunctionType.Gelu
            for m0, msz in chunks:
                # phase A: h^T[f, m] = gelu(W1^T @ x^T), kd-outer in groups
                # of FG PSUM banks
                hT = hpool.tile([P, FT, CHUNK_MAX], BF16)
                for g in range(FT // FG):
                    pss = [
                        pspool.tile(
                            [P, 512], F32, tag="a", bufs=FG + 2, name=f"ps_a_{j}"
                        )
                        for j in range(FG)
                    ]
                    for kd in range(KD):
                        for j in range(FG):
                            ft = g * FG + j
                            nc.tensor.matmul(
                                pss[j][:, :msz],
                                lhsT=w1sb[:, kd, ft * P : (ft + 1) * P],
                                rhs=xtsb[:, kd, m0 : m0 + msz],
                                start=(kd == 0),
                                stop=(kd == KD - 1),
                            )
                    for j in range(FG):
                        nc.scalar.activation(
                            hT[:, g * FG + j, :msz], pss[j][:, :msz], gelu
                        )

                # phase B: y^T[d, m] = W2^T-contraction over f (kf-inner)
                for dp in range(DP):
                    ps2 = pspool.tile([P, 512], F32, tag="b", bufs=2, name="ps_b")
                    for kf in range(FT):
                        nc.tensor.matmul(
                            ps2[:, :msz],
                            lhsT=w2sb[:, dp, kf, :],
                            rhs=hT[:, kf, :msz],
                            start=(kf == 0),
                            stop=(kf == FT - 1),
                        )
                    ysb = opool.tile([P, 512], F32)
                    nc.vector.tensor_copy(ysb[:, :msz], ps2[:, :msz])
                    nc.gpsimd.dma_start(
                        yt.ap()[dp * P : (dp + 1) * P, m0 : m0 + msz], ysb[:, :msz]
                    )
    nc.compile()
    return nc


def get_kernel(C: int, D: int, F: int):
    key = (C, D, F)
    if key not in _KERNEL_CACHE:
        _KERNEL_CACHE[key] = build_expert_ffn(C, D, F)
    return _KERNEL_CACHE[key]


# ---------------------------------------------------------------------------
# Host side: routing, sharding, gather/scatter
# ---------------------------------------------------------------------------


def _route(xf: np.ndarray, Wr: np.ndarray):
    """fp32 router identical to the reference: softmax + top-2 + renorm."""
    logits = xf @ Wr  # [N, E] fp32
    m = logits.max(axis=-1, keepdims=True)
    e = np.exp(logits - m)
    probs = (e / e.sum(axis=-1, keepdims=True)).astype(np.float32)
    # top-2, descending, ties -> lower index (jax.lax.top_k semantics)
    top_i = np.argsort(-probs, axis=-1, kind="stable")[:, :TOP_K]
    tw = np.take_along_axis(probs, top_i, axis=-1)
    top_w = (tw / tw.sum(axis=-1, keepdims=True)).astype(np.float32)
    return top_i, top_w


def kernel(x: np.ndarray, Wr: np.ndarray, W1: np.ndarray, W2: np.ndarray):
    global LAST_RESULTS
    x = np.ascontiguousarray(np.asarray(x, dtype=np.float32))
    Wr = np.ascontiguousarray(np.asarray(Wr, dtype=np.float32))
    W1 = np.ascontiguousarray(np.asarray(W1, dtype=np.float32))
    W2 = np.ascontiguousarray(np.asarray(W2, dtype=np.float32))

    B, T, D = x.shape
    E = Wr.shape[1]
    F = W1.shape[2]
    N = B * T
    xf = x.reshape(N, D)

    # --- router (host, fp32, replicated logic) ---
    top_i, top_w = _route(xf, Wr)

    # --- all-to-all by routed expert: build per-expert token lists ---
    idx_e = [np.where((top_i == e).any(axis=1))[0] for e in range(E)]
    w_e = []
    for e in range(E):
        sel = top_i[idx_e[e]] == e  # [n_e, 2] exactly one True per row
        w_e.append(
            np.where(sel[:, 0], top_w[idx_e[e], 0], top_w[idx_e[e], 1]).astype(
                np.float32
            )
        )
    counts = np.array([len(i) for i in idx_e])
    C = max(128, int(np.ceil(counts.max() / 64) * 64))  # capacity (static shape)

    KD = D // P
    FT = F // P

    # --- per-core input prep (gather + cast + device layout) ---
    in_maps = []
    for e in range(E):
        n_e = counts[e]
        xe = np.zeros((C, D), dtype=np.float32)
        xe[:n_e] = xf[idx_e[e]]
        # x^T arranged [128, KD, C]
        xt_host = np.ascontiguousarray(
            xe.T.reshape(KD, P, C).transpose(1, 0, 2)
        ).astype(NP_BF16)
        w1_host = np.ascontiguousarray(
            W1[e].reshape(KD, P, F).transpose(1, 0, 2)
        ).astype(NP_BF16)
        w2_host = np.ascontiguousarray(
            W2[e].reshape(FT, P, D // P, P).transpose(1, 2, 0, 3)
        ).astype(NP_BF16)
        in_maps.append({"xt": xt_host, "w1": w1_host, "w2": w2_host})

    # --- run on the 8 cores ---
    nc = get_kernel(C, D, F)
    res = run_bass_kernel_spmd(nc, in_maps, core_ids=list(range(N_CORES)))
    LAST_RESULTS = res

    # --- un-all-to-all: weighted scatter-add ---
    out = np.zeros((N, D), dtype=np.float32)
    for e in range(E):
        n_e = counts[e]
        if n_e:
            y = res.results[e]["yt"][:, :n_e].T  # [n_e, D]
            out[idx_e[e]] += w_e[e][:, None] * y

    return out.reshape(B, T, D), np.float32(0.0)


# revision 11
# speedup vs baseline: 1.1330x; 1.0054x over previous
"""MoE layer (top-2 of 8 experts) on 8 NeuronCores via expert parallelism.

Contract: kernel(**inputs) takes the FULL unsharded inputs
  x [2, 2048, 1024] f32, Wr [1024, 8] f32, W1 [8, 1024, 4096] f32,
  W2 [8, 4096, 1024] f32
and returns the FULL output (out [2, 2048, 1024] f32, aux_loss f32).

Strategy (expert parallelism, per the sharding hint):
- Host computes the (tiny) router: logits = x@Wr, softmax, top-2,
  normalized combine weights.  This decides the sharding: tokens are
  all-to-all'd (gathered) by routed expert on the host.
- Core e receives: its expert's W1[e]/W2[e] (cast bf16) and the tokens
  routed to expert e (x^T layout, cast bf16, padded to capacity C).
- Core e computes y^T = (gelu(x @ W1) @ W2)^T entirely on device:
    phase A: h^T[f, m] = gelu(sum_d W1[d, f] * x^T[d, m])  (PE + ACT)
    phase B: y^T[d, m] = sum_f W2[f, d] * h^T[f, m]        (PE)
  bf16 operands, fp32 PSUM accumulation, exact (erf) Gelu on fp32.
  Phase A runs contraction(kd)-outermost over groups of 4 PSUM banks so
  compute starts as soon as the first weight slices land (instead of
  waiting for all of W1).
- Host applies the combine weights and scatter-adds the per-expert
  outputs back into the full [4096, 1024] output (the "un-all-to-all").
"""

import os
import sys
import types

import numpy as np
import ml_dtypes

# ---------------------------------------------------------------------------
# Optional NTFF-profile hook for trace=True under axon.  concourse's
# run_bass_kernel_spmd imports antenv.axon_hooks unconditionally when tracing
# is requested; agent images lack that module, so synthesize it.
# ---------------------------------------------------------------------------


def _install_axon_hooks():
    if "antenv.axon_hooks" in sys.modules:
        return
    hook = None
    try:
        import trn_agent_boot.trn_boot as _tb

        so = "/opt/axon/libaxon_pjrt.so"
        if os.path.exists(so):
            hook = _tb._ntff_profile_via_ctypes(so)
    except Exception:
        hook = None
    m = types.ModuleType("antenv.axon_hooks")
    m.get_axon_ntff_profile_hook = lambda: hook
    m.set_axon_ntff_profile_hook = lambda h: None
    sys.modules["antenv.axon_hooks"] = m
    try:
        import antenv

        antenv.axon_hooks = m
    except Exception:
        pass


_install_axon_hooks()

import concourse.bass as bass  # noqa: E402
import concourse.tile as tile  # noqa: E402
from concourse import bacc, mybir  # noqa: E402
from concourse.bass_utils import run_bass_kernel_spmd  # noqa: E402

BF16 = mybir.dt.bfloat16
F32 = mybir.dt.float32
NP_BF16 = ml_dtypes.bfloat16

N_CORES = 8
TOP_K = 2
P = 128

# Results of the most recent device run (for the test harness).
LAST_RESULTS = None

_KERNEL_CACHE: dict = {}


def _chunks_of(C: int, maxc: int = 512, align: int = 64):
    """Split C into chunks <= maxc, multiples of align, first chunk largest
    (the first chunk races the weight DMA, so bigger = more slack)."""
    n = -(-C // maxc)
    sizes = [min(maxc, C)]
    rem = C - sizes[0]
    if rem:
        base = (rem // (n - 1)) // align * align
        tail = [base] * (n - 1)
        left = rem - base * (n - 1)
        i = 0
        while left:
            tail[i] += align
            left -= align
            i = (i + 1) % (n - 1)
        sizes += tail
    out, m0 = [], 0
    for s in sizes:
        out.append((m0, s))
        m0 += s
    return out


# ---------------------------------------------------------------------------
# Device kernel: y^T[d, m] = (gelu(x[m, :] @ W1) @ W2)^T   for one expert
# ---------------------------------------------------------------------------


def build_expert_ffn(C: int, D: int, F: int):
    """Build the per-core expert FFN Bass kernel for capacity C tokens."""
    assert C % 64 == 0 and D % P == 0 and F % P == 0
    KD = D // P          # k-tiles of matmul 1 (contraction over D)
    FT = F // P          # f-tiles = k-tiles of matmul 2 (contraction over F)
    DP = D // P          # 128-row output blocks of y^T
    FG = 4               # ft-group size (PSUM banks used by phase A)

    nc = bacc.Bacc(
        "TRN2",
        target_bir_lowering=False,
        debug=False,
        enable_asserts=False,
        num_devices=N_CORES,
    )
    xt = nc.dram_tensor("xt", [P, KD, C], BF16, kind="ExternalInput")
    w1 = nc.dram_tensor("w1", [P, KD, F], BF16, kind="ExternalInput")
    # W2 repacked per output-row-block: [p, dp, kf, q] = W2[kf*128+p, dp*128+q]
    w2 = nc.dram_tensor("w2", [P, DP, FT, P], BF16, kind="ExternalInput")
    yt = nc.dram_tensor("yt", [D, C], F32, kind="ExternalOutput")

    chunks = _chunks_of(C)
    CHUNK_MAX = max(s for _, s in chunks)
    W1_COLS = FG * P  # fine W1 DMA piece width (f-columns) for the early race

    with tile.TileContext(nc) as tc:
        with (
            tc.tile_pool(name="weights", bufs=1) as wpool,
            tc.tile_pool(name="acts", bufs=1) as apool,
            tc.tile_pool(name="h", bufs=1) as hpool,
            tc.tile_pool(name="psum", bufs=1, space="PSUM") as pspool,
            tc.tile_pool(name="out", bufs=4) as opool,
        ):
            xtsb = apool.tile([P, KD, C], BF16)
            w1sb = wpool.tile([P, KD, F], BF16)
            w2sb = wpool.tile([P, DP, FT, P], BF16)
            m0_0, msz_0 = chunks[0]

            # Early supply race: the first ft-group needs xt[:, kd, chunk0]
            # and w1[:, kd, 0:FG*P] for every kd.  Interleave them on two
            # issue engines (gpsimd=SWDGE, sync=HWDGE) so issue rate doesn't
            # serialize the supply.
            for kd in range(KD):
                nc.gpsimd.dma_start(
                    xtsb[:, kd, m0_0 : m0_0 + msz_0],
                    xt.ap()[:, kd, m0_0 : m0_0 + msz_0],
                )
                nc.sync.dma_start(
                    w1sb[:, kd, 0:W1_COLS], w1.ap()[:, kd, 0:W1_COLS]
                )
            for kd in range(KD):
                # fg1 pieces alternate across both issue engines to keep the
                # supply rate ahead of the second/third ft-group's consumption
                eng = nc.gpsimd if kd % 2 else nc.sync
                eng.dma_start(
                    w1sb[:, kd, W1_COLS : 2 * W1_COLS],
                    w1.ap()[:, kd, W1_COLS : 2 * W1_COLS],
                )
            # first W2 row-block early so phase B chunk0 can start on time
            nc.sync.dma_start(w2sb[:, 0], w2.ap()[:, 0])
            # coarse remainder of W1, W2 row-blocks interleaved
            w2_next = 1
            for f0 in range(2 * W1_COLS, F, 2 * W1_COLS):
                for kd in range(KD):
                    nc.sync.dma_start(
                        w1sb[:, kd, f0 : f0 + 2 * W1_COLS],
                        w1.ap()[:, kd, f0 : f0 + 2 * W1_COLS],
                    )
                for _ in range(2):
                    if w2_next < DP:
                        nc.sync.dma_start(w2sb[:, w2_next], w2.ap()[:, w2_next])
                        w2_next += 1
            while w2_next < DP:
                nc.sync.dma_start(w2sb[:, w2_next], w2.ap()[:, w2_next])
                w2_next += 1
            # later token chunks (needed much later)
            for m0, msz in chunks[1:]:
                nc.gpsimd.dma_start(
                    xtsb[:, :, m0 : m0 + msz], xt.ap()[:, :, m0 : m0 + msz]
                )

            gelu = mybir.ActivationFunctionType.Gelu
            for m0, msz in chunks:
                # phase A: h^T[f, m] = gelu(W1^T @ x^T), kd-outer in groups
                # of FG PSUM banks
                hT = hpool.tile([P, FT, CHUNK_MAX], BF16)
                for g in range(FT // FG):
                    pss = [
                        pspool.tile(
                            [P, 512], F32, tag="a", bufs=FG + 2, name=f"ps_a_{j}"
                        )
                        for j in range(FG)
                    ]
                    for kd in range(KD):
                        for j in range(FG):
                            ft = g * FG + j
                            nc.tensor.matmul(
                                pss[j][:, :msz],
                                lhsT=w1sb[:, kd, ft * P : (ft + 1) * P],
                                rhs=xtsb[:, kd, m0 : m0 + msz],
                                start=(kd == 0),
                                stop=(kd == KD - 1),
                            )
                    for j in range(FG):
                        nc.scalar.activation(
                            hT[:, g * FG + j, :msz], pss[j][:, :msz], gelu
                        )

                # phase B: y^T[d, m] = W2^T-contraction over f (kf-inner)
                for dp in range(DP):
                    ps2 = pspool.tile([P, 512], F32, tag="b", bufs=2, name="ps_b")
                    for kf in range(FT):
                        nc.tensor.matmul(
                            ps2[:, :msz],
                            lhsT=w2sb[:, dp, kf, :],
                            rhs=hT[:, kf, :msz],
                            start=(kf == 0),
                            stop=(kf == FT - 1),
                        )
                    ysb = opool.tile([P, 512], F32)
                    nc.vector.tensor_copy(ysb[:, :msz], ps2[:, :msz])
                    nc.sync.dma_start(
                        yt.ap()[dp * P : (dp + 1) * P, m0 : m0 + msz], ysb[:, :msz]
                    )
    nc.compile()
    return nc


def get_kernel(C: int, D: int, F: int):
    key = (C, D, F)
    if key not in _KERNEL_CACHE:
        _KERNEL_CACHE[key] = build_expert_ffn(C, D, F)
    return _KERNEL_CACHE[key]


# ---------------------------------------------------------------------------
# Host side: routing, sharding, gather/scatter
# ---------------------------------------------------------------------------


def _route(xf: np.ndarray, Wr: np.ndarray):
    """fp32 router identical to the reference: softmax + top-2 + renorm."""
    logits = xf @ Wr  # [N, E] fp32
    m = logits.max(axis=-1, keepdims=True)
    e = np.exp(logits - m)
    probs = (e / e.sum(axis=-1, keepdims=True)).astype(np.float32)
    # top-2, descending, ties -> lower index (jax.lax.top_k semantics)
    top_i = np.argsort(-probs, axis=-1, kind="stable")[:, :TOP_K]
    tw = np.take_along_axis(probs, top_i, axis=-1)
    top_w = (tw / tw.sum(axis=-1, keepdims=True)).astype(np.float32)
    return top_i, top_w


def kernel(x: np.ndarray, Wr: np.ndarray, W1: np.ndarray, W2: np.ndarray):
    global LAST_RESULTS
    x = np.ascontiguousarray(np.asarray(x, dtype=np.float32))
    Wr = np.ascontiguousarray(np.asarray(Wr, dtype=np.float32))
    W1 = np.ascontiguousarray(np.asarray(W1, dtype=np.float32))
    W2 = np.ascontiguousarray(np.asarray(W2, dtype=np.float32))

    B, T, D = x.shape
    E = Wr.shape[1]
    F = W1.shape[2]
    N = B * T
    xf = x.reshape(N, D)

    # --- router (host, fp32, replicated logic) ---
    top_i, top_w = _route(xf, Wr)

    # --- all-to-all by routed expert: build per-expert token lists ---
    idx_e = [np.where((top_i == e).any(axis=1))[0] for e in range(E)]
    w_e = []
    for e in range(E):
        sel = top_i[idx_e[e]] == e  # [n_e, 2] exactly one True per row
        w_e.append(
            np.where(sel[:, 0], top_w[idx_e[e], 0], top_w[idx_e[e], 1]).astype(
                np.float32
            )
        )
    counts = np.array([len(i) for i in idx_e])
    C = max(128, int(np.ceil(counts.max() / 64) * 64))  # capacity (static shape)

    KD = D // P
    FT = F // P

    # --- per-core input prep (gather + cast + device layout) ---
    in_maps = []
    for e in range(E):
        n_e = counts[e]
        xe = np.zeros((C, D), dtype=np.float32)
        xe[:n_e] = xf[idx_e[e]]
        # x^T arranged [128, KD, C]
        xt_host = np.ascontiguousarray(
            xe.T.reshape(KD, P, C).transpose(1, 0, 2)
        ).astype(NP_BF16)
        w1_host = np.ascontiguousarray(
            W1[e].reshape(KD, P, F).transpose(1, 0, 2)
        ).astype(NP_BF16)
        w2_host = np.ascontiguousarray(
            W2[e].reshape(FT, P, D // P, P).transpose(1, 2, 0, 3)
        ).astype(NP_BF16)
        in_maps.append({"xt": xt_host, "w1": w1_host, "w2": w2_host})

    # --- run on the 8 cores ---
    nc = get_kernel(C, D, F)
    res = run_bass_kernel_spmd(nc, in_maps, core_ids=list(range(N_CORES)))
    LAST_RESULTS = res

    # --- un-all-to-all: weighted scatter-add ---
    out = np.zeros((N, D), dtype=np.float32)
    for e in range(E):
        n_e = counts[e]
        if n_e:
            y = res.results[e]["yt"][:, :n_e].T  # [n_e, D]
            out[idx_e[e]] += w_e[e][:, None] * y

    return out.reshape(B, T, D), np.float32(0.0)
